# revision 3
# baseline (speedup 1.0000x reference)
"""MinGRU Trainium2 kernel (B=8, T=8192, D=H=512), SPMD over 8 NeuronCores.

v2 strategy (vs the 141 us baseline, whose trace showed PE busy 113.7 us and
DVE busy 112.7 us as co-bottlenecks):

  - Data-parallel over batch: core b computes batch row b end-to-end.
  - k-matmul (gate path) in fp8-e4m3 DoubleRow: weights 16*Wz and x both
    quantized to e4m3; each 128x512 output slice takes 2 MMs (contraction
    256/MM) instead of 4. The 1/16 descale folds into the ACT sigmoid scale.
    The th-matmul stays bf16 (4 MMs) -- e4m3 there pushes rel err past the
    2e-2 gate (gate-path errors are squashed by sigmoid's 0.25 slope;
    th errors pass through g~ at slope 1). PE: ~113.7 -> ~86 us.
  - 1 column/cycle scan (vs 2 for the stock/previous op): the DVE feedback
    (NEXT_ALU_OUT_A/B at the mult stage reading the add stage's flop) is one
    CYCLE deep, so at full rate element i sees element i-2's state. Feed the
    op a stream interleaving the two h-groups of a pair (column order
    g0[t], g1[t], g0[t+1], ...) and i-2 IS the same group's previous step.
    Two steady uops alternate per element (COUNT=1 triggers, zero-latency
    transitions): stream 0 keeps state in the stage-3 a-flop, stream 1 in the
    b-flop, so the recurrence is element-indexed and immune to input stalls.
    The interleave itself is pure access-pattern: tiles are [P, 2, TC+1]
    (group-major, contiguous), the scan reads/writes the "p a t -> p t a"
    rearranged view. DVE scan: ~68 -> ~34 us.
  - Chunk chaining without extra state plumbing: column 0 of each a-tile is
    pre-zeroed (once per pool buffer) and column 0 of each g-tile carries the
    previous chunk's last h (tiny GpSimd copy); a=0 makes h := g = h_prev
    exactly, re-seeding the recurrence from the data stream.
  - h stored/DMA'd as bf16 (halves output traffic; scan state stays f32 in
    the flops). a, s, g~ tiles bf16 (halves SBUF + DVE/ACT read traffic).
"""

import os
import sys

import numpy as np

if "/opt/trn_rl_repo" not in sys.path:
    sys.path.insert(0, "/opt/trn_rl_repo")

P = 128
B, T, D, H = 8, 8192, 512, 512
GD, GH = D // P, H // P  # 4, 4
TC = 1024  # time chunk; 2-bank PSUM tiles
NCORES = 8

_NC_CACHE = {}
LAST_RESULT = None  # BassKernelResults of the most recent run (for test.py)

_FSCAN_IL_OP = None


def _fscan_il_reference(in0, in1, c0, c1, c2):
    """CoreSim reference: 2-way interleaved scan. Stream order (free dims
    flattened) alternates two independent recurrences; both seeded from c0.
    h[p, j] = a[j]*state[p, j%2] + (1-a[j])*g[p, j]."""
    a = np.asarray(in0, np.float32)
    g = np.asarray(in1, np.float32)
    p = a.shape[0]
    fa = a.reshape(p, -1)
    fg = g.reshape(p, -1)
    if isinstance(c0, np.ndarray):
        init = c0.reshape(p).astype(np.float32)
    else:
        init = np.full(p, float(c0), np.float32)
    state = np.stack([init, init], axis=1)  # [p, 2]
    out = np.empty_like(fa)
    one = np.float32(1.0)
    for t in range(fa.shape[1]):
        s = t & 1
        st = fa[:, t] * state[:, s] + (one - fa[:, t]) * fg[:, t]
        out[:, t] = st
        state[:, s] = st
    return out.reshape(a.shape)


def register_fscan_il(variant="ab"):
    """Register MINGRU_FSCAN_IL: 1 elem/cycle 2-way interleaved scan.

    Datapath per element (input lanes: 1=a, 2=g~, 3=1.0f, 4=init):
      blk0: t = 1 - a
      blk1: u = t * g~
      blk2: m = a * state   (state: init chain for the first 2 elements;
                             then NEXT_ALU_OUT_A for even-indexed elements /
                             NEXT_ALU_OUT_B for odd -- each stream's state
                             persists in its own stage-3 flop)
      blk3: state' = m + u  (a-flop write on even uops, b-flop on odd)
      blk4-7: bypass; WR0 <- last block.
    FSM: initA (1 elem) -> initB (1 elem) -> steadyA <-> steadyB, all COUNT=1
    transitions (zero latency) => 1 element per cycle, element-indexed state
    selection (robust to input-stream stalls, unlike cycle-timed feedback).

    variant="2slice": fallback if the b-flop latches something other than the
    ALU result -- stream 1 instead computes at blocks 4/5 and keeps state in
    the stage-5 a-flop.
    """
    global _FSCAN_IL_OP
    if _FSCAN_IL_OP is not None:
        return _FSCAN_IL_OP

    from concourse.dve_ops import _SUB_OPCODE_FOR_NAME, CUSTOM_DVE_SPECS, OPS, DveOp
    from concourse.dve_spec import One, Spec, Src0, Src1
    from concourse.dve_uop import (
        ENABLE,
        AluInp,
        AluOp,
        DelayInp,
        DveOpSpec,
        InpSel,
        OutPath,
        OutSel,
        Trigger,
        UopConfig,
    )

    name = "MINGRU_FSCAN_IL"
    if name in _SUB_OPCODE_FOR_NAME:
        for op_ in OPS:
            if op_.name == name:
                _FSCAN_IL_OP = op_
                return op_

    # placeholder body (never lowered); reference drives CoreSim.
    spec = Spec(body=(One - Src0) * Src1, reference=_fscan_il_reference)

    def _uop(kind: str, nxt: int) -> UopConfig:
        u = UopConfig()
        lanes = (
            (1, InpSel.SRC_0),
            (2, InpSel.SRC_1),
            (3, InpSel.ONE_F32),
            (4, InpSel.CONST_0),
        )
        for lane, sel in lanes:
            u.inp[lane] = sel
            u.inp_enable[lane] = ENABLE
        dp = u.datapath_config
        # blk0: t = 1 - a; carry chains 0 (a), 1 (g~), 3 (init)
        dp[0].enable_alu(AluOp.SUBTRACT, AluInp.PREV_DELAY_2, AluInp.PREV_DELAY_0)
        dp[0].pass_through_delay(0, 1, 3)
        # blk1: u = t * g~; carry chains 0 (a), 3 (init)
        dp[1].enable_alu(AluOp.MULTIPLY, AluInp.PREV_ALU_OUT, AluInp.PREV_DELAY_1)
        dp[1].pass_through_delay(0, 3)
        if variant == "ab" or kind in ("initA", "stdA"):
            # compute at blocks 2/3
            if kind.startswith("init"):
                state_src = AluInp.PREV_DELAY_3
            elif kind == "stdA":
                state_src = AluInp.NEXT_ALU_OUT_A
            else:
                state_src = AluInp.NEXT_ALU_OUT_B
            dp[2].enable_alu(AluOp.MULTIPLY, AluInp.PREV_DELAY_0, state_src)
            dp[2].enable_delay_from_src(DelayInp.PREV_ALU_OUT, 2)
            dp[3].enable_alu(AluOp.ADD, AluInp.PREV_ALU_OUT, AluInp.PREV_DELAY_2)
            if variant == "ab" and kind in ("initB", "stdB"):
                dp[3].alu_out_b_enable = ENABLE
            else:
                dp[3].alu_out_a_enable = ENABLE
            for b in range(4, 8):
                dp[b].pass_through_alu()
        else:
            # 2slice variant, B-stream: compute at blocks 4/5, state in the
            # stage-5 a-flop (stage-3 a-flop belongs to the A stream).
            dp[2].pass_through_alu()
            dp[2].enable_delay_from_src(DelayInp.PREV_ALU_OUT, 2)
            dp[2].pass_through_delay(0, 3)
            dp[3].pass_through_alu()
            dp[3].pass_through_delay(0, 2, 3)
            state_src = (
                AluInp.PREV_DELAY_3 if kind == "initB" else AluInp.NEXT_ALU_OUT_A
            )
            dp[4].enable_alu(AluOp.MULTIPLY, AluInp.PREV_DELAY_0, state_src)
            dp[4].pass_through_delay(2)
            dp[5].enable_alu(AluOp.ADD, AluInp.PREV_ALU_OUT, AluInp.PREV_DELAY_2)
            dp[5].alu_out_a_enable = ENABLE
            for b in range(6, 8):
                dp[b].pass_through_alu()
        u.out[OutPath.WR0_LO] = OutSel.ALU_OUT
        u.out_enable[OutPath.WR0_LO] = ENABLE
        u.require_inp0 = ENABLE
        u.require_inp1 = ENABLE
        u.repeat_count = 1
        u.trigger = (Trigger.SRC_TENSOR_DONE, Trigger.COUNT, Trigger.NONE)
        u.next_uop = (0, nxt, 0)
        return u

    uops = [
        _uop("initA", 1),  # elem 0 -> initB
        _uop("initB", 2),  # elem 1 -> steadyA
        _uop("stdA", 3),  # even elems -> steadyB
        _uop("stdB", 2),  # odd elems -> steadyA
    ]
    for u in uops:
        u.validate("v3")

    row = max(_SUB_OPCODE_FOR_NAME.values()) + 1
    assert row < 0x20

    class _HandDveOp(DveOp):
        def compile(self, ver):
            from concourse.dve_ops import _COMPILE_CACHE

            key = (self.name, ver)
            if key in _COMPILE_CACHE:
                return _COMPILE_CACHE[key]
            assert ver == "v3", "MINGRU_FSCAN_IL is hand-authored for TRN2 (v3) only"
            r = DveOpSpec(name=self.name, opcode=row, uops=list(uops), rd1_en=True)
            _COMPILE_CACHE[key] = r
            return r

    op = _HandDveOp(name=name, spec=spec, subdim=False, uops_sha={})
    OPS.append(op)
    CUSTOM_DVE_SPECS[op.name] = spec
    _SUB_OPCODE_FOR_NAME[op.name] = row
    _FSCAN_IL_OP = op
    return op


def _build_nc(t_len=T, tc=TC, k8=True, variant="ab"):
    from contextlib import ExitStack

    import concourse.mybir as mybir
    import concourse.tile as tile
    from concourse import bacc

    f32 = mybir.dt.float32
    bf16 = mybir.dt.bfloat16
    fp8 = mybir.dt.float8e4
    Alu = mybir.AluOpType
    Act = mybir.ActivationFunctionType

    fscan_op = register_fscan_il(variant)

    nchunk = t_len // tc
    nc = bacc.Bacc("TRN2", target_bir_lowering=False, debug=False)

    xT = nc.dram_tensor("xT", [D, t_len], bf16, kind="ExternalInput").ap()
    x8 = nc.dram_tensor("x8", [D, t_len], fp8, kind="ExternalInput").ap()
    wz8T = nc.dram_tensor("wz8T", [D, H], fp8, kind="ExternalInput").ap()
    whT = nc.dram_tensor("whT", [D, H], bf16, kind="ExternalInput").ap()
    bias3 = nc.dram_tensor("bias3", [P, 3, GH], f32, kind="ExternalInput").ap()
    wzT = nc.dram_tensor("wzT", [D, H], bf16, kind="ExternalInput").ap()
    hT = nc.dram_tensor("hT", [H, t_len], bf16, kind="ExternalOutput").ap()

    xT_g = xT.rearrange("(g p) t -> p g t", p=P)
    x8_g = x8.rearrange("(g p) t -> p g t", p=P)
    hT_g = hT.rearrange("(g p) t -> p g t", p=P)

    kscale = -1.0 / 16.0 if k8 else -1.0

    with tile.TileContext(nc) as tctx, ExitStack() as ctx:
        singles = ctx.enter_context(tctx.tile_pool(name="singles", bufs=1))
        xpool = ctx.enter_context(tctx.tile_pool(name="xp", bufs=3))
        x8pool = ctx.enter_context(tctx.tile_pool(name="x8p", bufs=3))
        hpool = ctx.enter_context(tctx.tile_pool(name="hp", bufs=3))
        apool = ctx.enter_context(tctx.tile_pool(name="apool", bufs=4))
        spool = ctx.enter_context(tctx.tile_pool(name="spool", bufs=5))
        gpool = ctx.enter_context(tctx.tile_pool(name="gpool", bufs=4))
        kp = ctx.enter_context(tctx.tile_pool(name="kp", bufs=2, space="PSUM"))
        tp = ctx.enter_context(tctx.tile_pool(name="tp", bufs=2, space="PSUM"))

        wz8_r = wz8T.rearrange("(g p) h -> p g h", p=P)
        wzT_r = wzT.rearrange("(g p) h -> p g h", p=P)
        whT_r = whT.rearrange("(g p) h -> p g h", p=P)
        # biases first (tiny DMA, ungates the first ACTIVATE), then weights on
        # the scalar queue while x streams on sync.
        bias3_sb = singles.tile([P, 3, GH], f32)
        nc.scalar.dma_start(out=bias3_sb, in_=bias3)
        if k8:
            wz_sb = singles.tile([P, GD, H], fp8)
            nc.scalar.dma_start(out=wz_sb, in_=wz8_r)
        else:
            wz_sb = singles.tile([P, GD, H], bf16)
            nc.scalar.dma_start(out=wz_sb, in_=wzT_r)
        wh_sb = singles.tile([P, GD, H], bf16)
        nc.scalar.dma_start(out=wh_sb, in_=whT_r)

        # Pre-zero the seed column (t=0) of every a-tile buffer: ACT only ever
        # writes columns 1.., pool rotation preserves the zeros, and a=0 turns
        # the seed column into h := g (the chunk-chaining re-seed). g-tile
        # seed columns start at 0 (chunk 0's h_{-1}); later chunks overwrite
        # them with the previous chunk's last h.
        a_bufs, g_bufs = {}, {}
        for pair in range(2):
            for i in range(4):
                at = apool.tile([P, 2, tc + 1], bf16, tag=f"a{pair}")
                nc.vector.memset(at[:, :, 0:1], 0.0)
                a_bufs[(pair, i)] = at
                gt = gpool.tile([P, 2, tc + 1], bf16, tag=f"g{pair}")
                nc.vector.memset(gt[:, :, 0:1], 0.0)
                g_bufs[(pair, i)] = gt

        h_prev = {}
        c_off = 0
        for c in range(nchunk):
            nh = tc // 2  # x DMA halves: finer DMA/dependency granularity
            x_halves, x8_halves = [], []
            for hidx in range(tc // nh):
                sl = slice(c_off + hidx * nh, c_off + (hidx + 1) * nh)
                xh = xpool.tile([P, GD, nh], bf16, tag=f"x{hidx}")
                nc.sync.dma_start(out=xh, in_=xT_g[:, :, sl])
                x_halves.append(xh)
                if k8:
                    x8h = x8pool.tile([P, GD, nh], fp8, tag=f"x8{hidx}")
                    nc.sync.dma_start(out=x8h, in_=x8_g[:, :, sl])
                    x8_halves.append(x8h)
                else:
                    x8_halves.append(xh)

            for g in range(GH):
                pair, slot = g // 2, g % 2
                a_il = a_bufs[(pair, c % 4)]
                g_il = g_bufs[(pair, c % 4)]
                kps = kp.tile([P, tc], f32, tag="k")
                tps = tp.tile([P, tc], f32, tag="t")
                nw = 512
                for ns in range(tc // nw):
                    nsl = slice(ns * nw, (ns + 1) * nw)
                    xh = x_halves[(ns * nw) // nh]
                    xsl = slice((ns * nw) % nh, (ns * nw) % nh + nw)
                    for gd in range(GD):
                        nc.tensor.matmul(
                            tps[:, nsl],
                            wh_sb[:, gd, g * P : (g + 1) * P],
                            xh[:, gd, xsl],
                            start=(gd == 0),
                            stop=(gd == GD - 1),
                        )
                for ns in range(tc // nw):
                    nsl = slice(ns * nw, (ns + 1) * nw)
                    x8h = x8_halves[(ns * nw) // nh]
                    xsl = slice((ns * nw) % nh, (ns * nw) % nh + nw)
                    if k8:
                        for j in range(2):
                            nc.tensor.matmul(
                                kps[:, nsl],
                                wz_sb[:, 2 * j : 2 * j + 2, g * P : (g + 1) * P],
                                x8h[:, 2 * j : 2 * j + 2, xsl],
                                start=(j == 0),
                                stop=(j == 1),
                                perf_mode=mybir.MatmulPerfMode.DoubleRow,
                            )
                    else:
                        for gd in range(GD):
                            nc.tensor.matmul(
                                kps[:, nsl],
                                wz_sb[:, gd, g * P : (g + 1) * P],
                                x8h[:, gd, xsl],
                                start=(gd == 0),
                                stop=(gd == GD - 1),
                            )
                # s = sigmoid(th_mm + bh) -- first: g~ depends on it
                s_sb = spool.tile([P, tc], bf16, tag="s")
                nc.scalar.activation(
                    out=s_sb,
                    in_=tps,
                    func=Act.Sigmoid,
                    bias=bias3_sb[:, 1, g : g + 1],
                    scale=1.0,
                )
                # g~ = max(th_mm + (bh+0.5), s), into the pair-interleaved tile
                nc.vector.scalar_tensor_tensor(
                    out=g_il[:, slot, 1:],
                    in0=tps,
                    scalar=bias3_sb[:, 2, g : g + 1],
                    in1=s_sb,
                    op0=Alu.add,
                    op1=Alu.max,
                )
                # a = sigmoid(-(k_mm/16 + bz)) = Sigmoid(k_mm * -1/16 + (-bz))
                nc.scalar.activation(
                    out=a_il[:, slot, 1:],
                    in_=kps,
                    func=Act.Sigmoid,
                    bias=bias3_sb[:, 0, g : g + 1],
                    scale=kscale,
                )
                if slot == 1:
                    # pair complete: chain seed, scan, store
                    if c > 0:
                        nc.gpsimd.tensor_copy(
                            out=g_il[:, :, 0:1],
                            in_=h_prev[pair][:, :, tc : tc + 1],
                        )
                    h_il = hpool.tile([P, 2, tc + 1], bf16, tag=f"h{pair}")
                    nc.vector._custom_dve(
                        fscan_op,
                        out=h_il.rearrange("p a t -> p t a"),
                        in0=a_il.rearrange("p a t -> p t a"),
                        in1=g_il.rearrange("p a t -> p t a"),
                        s0=0.0,
                    )
                    for gg in range(2):
                        nc.sync.dma_start(
                            out=hT_g[:, 2 * pair + gg, c_off : c_off + tc],
                            in_=h_il[:, gg, 1:],
                        )
                    h_prev[pair] = h_il
            c_off += tc
    nc.compile()
    return nc


def get_nc(t_len=T, tc=TC, k8=True, variant="ab"):
    key = (t_len, tc, k8, variant)
    if key not in _NC_CACHE:
        _NC_CACHE[key] = _build_nc(t_len, tc, k8, variant)
    return _NC_CACHE[key]


def _prep_shared(Wz, bz, Wh, bh):
    import ml_dtypes

    f = np.float32
    bf = np.dtype(ml_dtypes.bfloat16)
    f8 = np.dtype(ml_dtypes.float8_e4m3)
    return {
        "wz8T": np.ascontiguousarray(Wz.T * np.float32(16.0)).astype(f8),
        "wzT": np.ascontiguousarray(Wz.T).astype(bf),
        "whT": np.ascontiguousarray(Wh.T).astype(bf),
        "bias3": np.ascontiguousarray(
            np.stack(
                [(-bz).reshape(GH, P).T, bh.reshape(GH, P).T, (bh + 0.5).reshape(GH, P).T],
                axis=1,
            ),
            dtype=f,
        ),
    }


def kernel(x, Wz, bz, Wh, bh):
    global LAST_RESULT
    import ml_dtypes

    from concourse import bass_utils

    x = np.asarray(x, dtype=np.float32)
    assert x.shape == (B, T, D), x.shape

    tc = int(os.environ.get("MINGRU_TC", str(TC)))
    k8 = os.environ.get("MINGRU_K8", "1") == "1"
    variant = os.environ.get("MINGRU_SCAN_VARIANT", "ab")
    nc = get_nc(tc=tc, k8=k8, variant=variant)
    shared = _prep_shared(
        np.asarray(Wz, np.float32),
        np.asarray(bz, np.float32),
        np.asarray(Wh, np.float32),
        np.asarray(bh, np.float32),
    )
    bf = np.dtype(ml_dtypes.bfloat16)
    f8 = np.dtype(ml_dtypes.float8_e4m3)
    in_maps = []
    for b in range(NCORES):
        xbT = np.ascontiguousarray(x[b].T)
        m = {"xT": xbT.astype(bf), "x8": xbT.astype(f8)}
        m.update(shared)
        in_maps.append(m)

    res = bass_utils.run_bass_kernel_spmd(
        nc,
        in_maps,
        core_ids=list(range(NCORES)),
        trace=os.environ.get("MINGRU_TRACE", "0") == "1",
    )
    LAST_RESULT = res
    out = np.stack(
        [res.results[b]["hT"].astype(np.float32).T for b in range(NCORES)]
    )
    return np.ascontiguousarray(out, dtype=np.float32)


# revision 7
# speedup vs baseline: 1.9373x; 1.9373x over previous
"""MinGRU Trainium2 kernel (B=8, T=8192, D=H=512), SPMD over 8 NeuronCores.

v2 strategy (vs the 141 us baseline, whose trace showed PE busy 113.7 us and
DVE busy 112.7 us as co-bottlenecks):

  - Data-parallel over batch: core b computes batch row b end-to-end.
  - k-matmul (gate path) in fp8-e4m3 DoubleRow: weights 16*Wz and x both
    quantized to e4m3; each 128x512 output slice takes 2 MMs (contraction
    256/MM) instead of 4. The 1/16 descale folds into the ACT sigmoid scale.
    The th-matmul stays bf16 (4 MMs) -- e4m3 there pushes rel err past the
    2e-2 gate (gate-path errors are squashed by sigmoid's 0.25 slope;
    th errors pass through g~ at slope 1). PE: ~113.7 -> ~86 us.
  - 1 column/cycle scan (vs 2 for the stock/previous op): the DVE feedback
    (NEXT_ALU_OUT_A/B at the mult stage reading the add stage's flop) is one
    CYCLE deep, so at full rate element i sees element i-2's state. Feed the
    op a stream interleaving the two h-groups of a pair (column order
    g0[t], g1[t], g0[t+1], ...) and i-2 IS the same group's previous step.
    Two steady uops alternate per element (COUNT=1 triggers, zero-latency
    transitions): stream 0 keeps state in the stage-3 a-flop, stream 1 in the
    b-flop, so the recurrence is element-indexed and immune to input stalls.
    The interleave itself is pure access-pattern: tiles are [P, 2, TC+1]
    (group-major, contiguous), the scan reads/writes the "p a t -> p t a"
    rearranged view. DVE scan: ~68 -> ~34 us.
  - Chunk chaining without extra state plumbing: column 0 of each a-tile is
    pre-zeroed (once per pool buffer) and column 0 of each g-tile carries the
    previous chunk's last h (tiny GpSimd copy); a=0 makes h := g = h_prev
    exactly, re-seeding the recurrence from the data stream.
  - h stored/DMA'd as bf16 (halves output traffic; scan state stays f32 in
    the flops). a, s, g~ tiles bf16 (halves SBUF + DVE/ACT read traffic).
"""

import os
import sys

import numpy as np

if "/opt/trn_rl_repo" not in sys.path:
    sys.path.insert(0, "/opt/trn_rl_repo")

P = 128
B, T, D, H = 8, 8192, 512, 512
GD, GH = D // P, H // P  # 4, 4
TC = 1024  # time chunk; 2-bank PSUM tiles
NCORES = 8

_NC_CACHE = {}
LAST_RESULT = None  # BassKernelResults of the most recent run (for test.py)

_FSCAN_IL_OP = None


def _fscan_il_reference(in0, in1, c0, c1, c2):
    """CoreSim reference: 2-way interleaved scan. Stream order (free dims
    flattened) alternates two independent recurrences; both seeded from c0.
    h[p, j] = a[j]*state[p, j%2] + (1-a[j])*g[p, j]."""
    a = np.asarray(in0, np.float32)
    g = np.asarray(in1, np.float32)
    p = a.shape[0]
    fa = a.reshape(p, -1)
    fg = g.reshape(p, -1)
    if isinstance(c0, np.ndarray):
        init = c0.reshape(p).astype(np.float32)
    else:
        init = np.full(p, float(c0), np.float32)
    state = np.stack([init, init], axis=1)  # [p, 2]
    out = np.empty_like(fa)
    one = np.float32(1.0)
    for t in range(fa.shape[1]):
        s = t & 1
        st = fa[:, t] * state[:, s] + (one - fa[:, t]) * fg[:, t]
        out[:, t] = st
        state[:, s] = st
    return out.reshape(a.shape)


def register_fscan_il(variant="ab"):
    """Register MINGRU_FSCAN_IL: 1 elem/cycle 2-way interleaved scan.

    Datapath per element (input lanes: 1=a, 2=g~, 3=1.0f, 4=init):
      blk0: t = 1 - a
      blk1: u = t * g~
      blk2: m = a * state   (state: init chain for the first 2 elements;
                             then NEXT_ALU_OUT_A for even-indexed elements /
                             NEXT_ALU_OUT_B for odd -- each stream's state
                             persists in its own stage-3 flop)
      blk3: state' = m + u  (a-flop write on even uops, b-flop on odd)
      blk4-7: bypass; WR0 <- last block.
    FSM: initA (1 elem) -> initB (1 elem) -> steadyA <-> steadyB, all COUNT=1
    transitions (zero latency) => 1 element per cycle, element-indexed state
    selection (robust to input-stream stalls, unlike cycle-timed feedback).

    variant="2slice": fallback if the b-flop latches something other than the
    ALU result -- stream 1 instead computes at blocks 4/5 and keeps state in
    the stage-5 a-flop.
    """
    global _FSCAN_IL_OP
    if _FSCAN_IL_OP is not None:
        return _FSCAN_IL_OP

    from concourse.dve_ops import _SUB_OPCODE_FOR_NAME, CUSTOM_DVE_SPECS, OPS, DveOp
    from concourse.dve_spec import One, Spec, Src0, Src1
    from concourse.dve_uop import (
        ENABLE,
        AluInp,
        AluOp,
        DelayInp,
        DveOpSpec,
        InpSel,
        OutPath,
        OutSel,
        Trigger,
        UopConfig,
    )

    name = "MINGRU_FSCAN_IL"
    if name in _SUB_OPCODE_FOR_NAME:
        for op_ in OPS:
            if op_.name == name:
                _FSCAN_IL_OP = op_
                return op_

    # placeholder body (never lowered); reference drives CoreSim.
    spec = Spec(body=(One - Src0) * Src1, reference=_fscan_il_reference)

    def _uop(kind: str, nxt: int) -> UopConfig:
        u = UopConfig()
        lanes = (
            (1, InpSel.SRC_0),
            (2, InpSel.SRC_1),
            (3, InpSel.ONE_F32),
            (4, InpSel.CONST_0),
        )
        for lane, sel in lanes:
            u.inp[lane] = sel
            u.inp_enable[lane] = ENABLE
        dp = u.datapath_config
        # blk0: t = 1 - a; carry chains 0 (a), 1 (g~), 3 (init)
        dp[0].enable_alu(AluOp.SUBTRACT, AluInp.PREV_DELAY_2, AluInp.PREV_DELAY_0)
        dp[0].pass_through_delay(0, 1, 3)
        # blk1: u = t * g~; carry chains 0 (a), 3 (init)
        dp[1].enable_alu(AluOp.MULTIPLY, AluInp.PREV_ALU_OUT, AluInp.PREV_DELAY_1)
        dp[1].pass_through_delay(0, 3)
        if variant == "ab" or kind in ("initA", "stdA"):
            # compute at blocks 2/3
            if kind.startswith("init"):
                state_src = AluInp.PREV_DELAY_3
            elif kind == "stdA":
                state_src = AluInp.NEXT_ALU_OUT_A
            else:
                state_src = AluInp.NEXT_ALU_OUT_B
            dp[2].enable_alu(AluOp.MULTIPLY, AluInp.PREV_DELAY_0, state_src)
            dp[2].enable_delay_from_src(DelayInp.PREV_ALU_OUT, 2)
            dp[3].enable_alu(AluOp.ADD, AluInp.PREV_ALU_OUT, AluInp.PREV_DELAY_2)
            if variant == "ab" and kind in ("initB", "stdB"):
                dp[3].alu_out_b_enable = ENABLE
            else:
                dp[3].alu_out_a_enable = ENABLE
            for b in range(4, 8):
                dp[b].pass_through_alu()
        else:
            # 2slice variant, B-stream: compute at blocks 4/5, state in the
            # stage-5 a-flop (stage-3 a-flop belongs to the A stream).
            dp[2].pass_through_alu()
            dp[2].enable_delay_from_src(DelayInp.PREV_ALU_OUT, 2)
            dp[2].pass_through_delay(0, 3)
            dp[3].pass_through_alu()
            dp[3].pass_through_delay(0, 2, 3)
            state_src = (
                AluInp.PREV_DELAY_3 if kind == "initB" else AluInp.NEXT_ALU_OUT_A
            )
            dp[4].enable_alu(AluOp.MULTIPLY, AluInp.PREV_DELAY_0, state_src)
            dp[4].pass_through_delay(2)
            dp[5].enable_alu(AluOp.ADD, AluInp.PREV_ALU_OUT, AluInp.PREV_DELAY_2)
            dp[5].alu_out_a_enable = ENABLE
            for b in range(6, 8):
                dp[b].pass_through_alu()
        u.out[OutPath.WR0_LO] = OutSel.ALU_OUT
        u.out_enable[OutPath.WR0_LO] = ENABLE
        u.require_inp0 = ENABLE
        u.require_inp1 = ENABLE
        u.repeat_count = 1
        u.trigger = (Trigger.SRC_TENSOR_DONE, Trigger.COUNT, Trigger.NONE)
        u.next_uop = (0, nxt, 0)
        return u

    uops = [
        _uop("initA", 1),  # elem 0 -> initB
        _uop("initB", 2),  # elem 1 -> steadyA
        _uop("stdA", 3),  # even elems -> steadyB
        _uop("stdB", 2),  # odd elems -> steadyA
    ]
    for u in uops:
        u.validate("v3")

    row = max(_SUB_OPCODE_FOR_NAME.values()) + 1
    assert row < 0x20

    class _HandDveOp(DveOp):
        def compile(self, ver):
            from concourse.dve_ops import _COMPILE_CACHE

            key = (self.name, ver)
            if key in _COMPILE_CACHE:
                return _COMPILE_CACHE[key]
            assert ver == "v3", "MINGRU_FSCAN_IL is hand-authored for TRN2 (v3) only"
            r = DveOpSpec(name=self.name, opcode=row, uops=list(uops), rd1_en=True)
            _COMPILE_CACHE[key] = r
            return r

    op = _HandDveOp(name=name, spec=spec, subdim=False, uops_sha={})
    OPS.append(op)
    CUSTOM_DVE_SPECS[op.name] = spec
    _SUB_OPCODE_FOR_NAME[op.name] = row
    _FSCAN_IL_OP = op
    return op


def _build_nc(t_len=T, tc=TC, k8=True, variant="ab"):
    from contextlib import ExitStack

    import concourse.mybir as mybir
    import concourse.tile as tile
    from concourse import bacc

    f32 = mybir.dt.float32
    bf16 = mybir.dt.bfloat16
    fp8 = mybir.dt.float8e4
    Alu = mybir.AluOpType
    Act = mybir.ActivationFunctionType

    fscan_op = register_fscan_il(variant)

    nchunk = t_len // tc
    nc = bacc.Bacc("TRN2", target_bir_lowering=False, debug=False)

    xT = nc.dram_tensor("xT", [D, t_len], bf16, kind="ExternalInput").ap()
    x8 = nc.dram_tensor("x8", [D, t_len], fp8, kind="ExternalInput").ap()
    wz8T = nc.dram_tensor("wz8T", [D, H], fp8, kind="ExternalInput").ap()
    whT = nc.dram_tensor("whT", [D, H], bf16, kind="ExternalInput").ap()
    bias3 = nc.dram_tensor("bias3", [P, 3, GH], f32, kind="ExternalInput").ap()
    wzT = nc.dram_tensor("wzT", [D, H], bf16, kind="ExternalInput").ap()
    # h output stays pair-interleaved: [pair, p, t, j] = h[(2*pair+j)*128+p, t]
    # (the scan writes time-major interleaved pairs; host de-interleaves)
    hT2 = nc.dram_tensor("hT2", [2, P, t_len, 2], bf16, kind="ExternalOutput").ap()

    xT_g = xT.rearrange("(g p) t -> p g t", p=P)
    x8_g = x8.rearrange("(g p) t -> p g t", p=P)

    kscale = -1.0 / 16.0 if k8 else -1.0

    with tile.TileContext(nc) as tctx, ExitStack() as ctx:
        singles = ctx.enter_context(tctx.tile_pool(name="singles", bufs=1))
        xpool = ctx.enter_context(tctx.tile_pool(name="xp", bufs=3))
        x8pool = ctx.enter_context(tctx.tile_pool(name="x8p", bufs=3))
        hpool = ctx.enter_context(tctx.tile_pool(name="hp", bufs=3))
        apool = ctx.enter_context(tctx.tile_pool(name="apool", bufs=4))
        spool = ctx.enter_context(tctx.tile_pool(name="spool", bufs=5))
        gpool = ctx.enter_context(tctx.tile_pool(name="gpool", bufs=4))
        kp = ctx.enter_context(tctx.tile_pool(name="kp", bufs=2, space="PSUM"))
        tp = ctx.enter_context(tctx.tile_pool(name="tp", bufs=2, space="PSUM"))

        wz8_r = wz8T.rearrange("(g p) h -> p g h", p=P)
        wzT_r = wzT.rearrange("(g p) h -> p g h", p=P)
        whT_r = whT.rearrange("(g p) h -> p g h", p=P)
        # biases first (tiny DMA, ungates the first ACTIVATE), then weights on
        # the scalar queue while x streams on sync.
        bias3_sb = singles.tile([P, 3, GH], f32)
        nc.scalar.dma_start(out=bias3_sb, in_=bias3)
        if k8:
            wz_sb = singles.tile([P, GD, H], fp8)
            nc.scalar.dma_start(out=wz_sb, in_=wz8_r)
        else:
            wz_sb = singles.tile([P, GD, H], bf16)
            nc.scalar.dma_start(out=wz_sb, in_=wzT_r)
        wh_sb = singles.tile([P, GD, H], bf16)
        nc.scalar.dma_start(out=wh_sb, in_=whT_r)

        # Pre-zero the seed column (t=0) of every a-tile buffer: ACT only ever
        # writes columns 1.., pool rotation preserves the zeros, and a=0 turns
        # the seed column into h := g (the chunk-chaining re-seed). g-tile
        # seed columns start at 0 (chunk 0's h_{-1}); later chunks overwrite
        # them with the previous chunk's last h.
        a_bufs, g_bufs = {}, {}
        for pair in range(2):
            for i in range(4):
                at = apool.tile([P, tc + 1, 2], bf16, tag=f"a{pair}")
                nc.vector.memset(at[:, 0:1, :], 0.0)
                a_bufs[(pair, i)] = at
                gt = gpool.tile([P, tc + 1, 2], bf16, tag=f"g{pair}")
                nc.vector.memset(gt[:, 0:1, :], 0.0)
                g_bufs[(pair, i)] = gt

        h_prev = {}
        c_off = 0
        for c in range(nchunk):
            nh = tc // 2  # x DMA halves: finer DMA/dependency granularity
            x_halves, x8_halves = [], []
            for hidx in range(tc // nh):
                sl = slice(c_off + hidx * nh, c_off + (hidx + 1) * nh)
                xh = xpool.tile([P, GD, nh], bf16, tag=f"x{hidx}")
                nc.sync.dma_start(out=xh, in_=xT_g[:, :, sl])
                x_halves.append(xh)
                if k8:
                    x8h = x8pool.tile([P, GD, nh], fp8, tag=f"x8{hidx}")
                    nc.sync.dma_start(out=x8h, in_=x8_g[:, :, sl])
                    x8_halves.append(x8h)
                else:
                    x8_halves.append(xh)

            for g in range(GH):
                pair, slot = g // 2, g % 2
                a_il = a_bufs[(pair, c % 4)]
                g_il = g_bufs[(pair, c % 4)]
                kps = kp.tile([P, tc], f32, tag="k")
                tps = tp.tile([P, tc], f32, tag="t")
                nw = 512
                for ns in range(tc // nw):
                    nsl = slice(ns * nw, (ns + 1) * nw)
                    xh = x_halves[(ns * nw) // nh]
                    xsl = slice((ns * nw) % nh, (ns * nw) % nh + nw)
                    for gd in range(GD):
                        nc.tensor.matmul(
                            tps[:, nsl],
                            wh_sb[:, gd, g * P : (g + 1) * P],
                            xh[:, gd, xsl],
                            start=(gd == 0),
                            stop=(gd == GD - 1),
                        )
                for ns in range(tc // nw):
                    nsl = slice(ns * nw, (ns + 1) * nw)
                    x8h = x8_halves[(ns * nw) // nh]
                    xsl = slice((ns * nw) % nh, (ns * nw) % nh + nw)
                    if k8:
                        for j in range(2):
                            nc.tensor.matmul(
                                kps[:, nsl],
                                wz_sb[:, 2 * j : 2 * j + 2, g * P : (g + 1) * P],
                                x8h[:, 2 * j : 2 * j + 2, xsl],
                                start=(j == 0),
                                stop=(j == 1),
                                perf_mode=mybir.MatmulPerfMode.DoubleRow,
                            )
                    else:
                        for gd in range(GD):
                            nc.tensor.matmul(
                                kps[:, nsl],
                                wz_sb[:, gd, g * P : (g + 1) * P],
                                x8h[:, gd, xsl],
                                start=(gd == 0),
                                stop=(gd == GD - 1),
                            )
                # s = sigmoid(th_mm + bh) -- first: g~ depends on it
                s_sb = spool.tile([P, tc], bf16, tag="s")
                nc.scalar.activation(
                    out=s_sb,
                    in_=tps,
                    func=Act.Sigmoid,
                    bias=bias3_sb[:, 1, g : g + 1],
                    scale=1.0,
                )
                # g~ = max(th_mm + (bh+0.5), s), into the pair-interleaved tile
                nc.vector.scalar_tensor_tensor(
                    out=g_il[:, 1:, slot],
                    in0=tps,
                    scalar=bias3_sb[:, 2, g : g + 1],
                    in1=s_sb,
                    op0=Alu.add,
                    op1=Alu.max,
                )
                # a = sigmoid(-(k_mm/16 + bz)) = Sigmoid(k_mm * -1/16 + (-bz))
                nc.scalar.activation(
                    out=a_il[:, 1:, slot],
                    in_=kps,
                    func=Act.Sigmoid,
                    bias=bias3_sb[:, 0, g : g + 1],
                    scale=kscale,
                )
                if slot == 1:
                    # pair complete: chain seed, scan, store
                    if c > 0:
                        nc.gpsimd.tensor_copy(
                            out=g_il[:, 0:1, :],
                            in_=h_prev[pair][:, tc : tc + 1, :],
                        )
                    h_il = hpool.tile([P, tc + 1, 2], bf16, tag=f"h{pair}")
                    nc.vector._custom_dve(
                        fscan_op,
                        out=h_il,
                        in0=a_il,
                        in1=g_il,
                        s0=0.0,
                    )
                    nc.sync.dma_start(
                        out=hT2[pair, :, c_off : c_off + tc, :],
                        in_=h_il[:, 1:, :],
                    )
                    h_prev[pair] = h_il
            c_off += tc
    nc.compile()
    return nc


def get_nc(t_len=T, tc=TC, k8=True, variant="ab"):
    key = (t_len, tc, k8, variant)
    if key not in _NC_CACHE:
        _NC_CACHE[key] = _build_nc(t_len, tc, k8, variant)
    return _NC_CACHE[key]


def _prep_shared(Wz, bz, Wh, bh):
    import ml_dtypes

    f = np.float32
    bf = np.dtype(ml_dtypes.bfloat16)
    f8 = np.dtype(ml_dtypes.float8_e4m3)
    return {
        "wz8T": np.ascontiguousarray(Wz.T * np.float32(16.0)).astype(f8),
        "wzT": np.ascontiguousarray(Wz.T).astype(bf),
        "whT": np.ascontiguousarray(Wh.T).astype(bf),
        "bias3": np.ascontiguousarray(
            np.stack(
                [(-bz).reshape(GH, P).T, bh.reshape(GH, P).T, (bh + 0.5).reshape(GH, P).T],
                axis=1,
            ),
            dtype=f,
        ),
    }


def kernel(x, Wz, bz, Wh, bh):
    global LAST_RESULT
    import ml_dtypes

    from concourse import bass_utils

    x = np.asarray(x, dtype=np.float32)
    assert x.shape == (B, T, D), x.shape

    tc = int(os.environ.get("MINGRU_TC", str(TC)))
    k8 = os.environ.get("MINGRU_K8", "1") == "1"
    variant = os.environ.get("MINGRU_SCAN_VARIANT", "ab")
    nc = get_nc(tc=tc, k8=k8, variant=variant)
    shared = _prep_shared(
        np.asarray(Wz, np.float32),
        np.asarray(bz, np.float32),
        np.asarray(Wh, np.float32),
        np.asarray(bh, np.float32),
    )
    bf = np.dtype(ml_dtypes.bfloat16)
    f8 = np.dtype(ml_dtypes.float8_e4m3)
    in_maps = []
    for b in range(NCORES):
        xbT = np.ascontiguousarray(x[b].T)
        m = {"xT": xbT.astype(bf), "x8": xbT.astype(f8)}
        m.update(shared)
        in_maps.append(m)

    res = bass_utils.run_bass_kernel_spmd(
        nc,
        in_maps,
        core_ids=list(range(NCORES)),
        trace=os.environ.get("MINGRU_TRACE", "0") == "1",
    )
    LAST_RESULT = res
    outs = []
    for b in range(NCORES):
        h2 = res.results[b]["hT2"].astype(np.float32)  # [2, 128, T, 2]
        # h channel (2*pair+j)*128 + p at time t lives at h2[pair, p, t, j]
        o = h2.transpose(0, 3, 1, 2).reshape(H, T).T  # [T, H]
        outs.append(o)
    return np.ascontiguousarray(np.stack(outs), dtype=np.float32)
